# revision 13
# baseline (speedup 1.0000x reference)
"""Trainium2 Bass kernel for nn_ASIS_46420006535338 (retrieval_knn).

Pipeline per batch (one batch per NeuronCore, 8 cores):
  adapted = relu(W_eff @ f_sem + b_eff)            # BN folded into conv weights
  f_sins  = f_ins + adapted
  e       = W_ins @ f_sins                         # [5, N] (bias cancels in distances)
  neg_dist[n, m] = 2 e_n.e_m - |e_n|^2 - |e_m|^2   # via one 7-row PE matmul
  top-30 by neg_dist per row -> indices            # DVE max/max_index/match_replace
  f_isem[:, n] = max_k f_sem[:, idx[n, k]]         # bulk dma_gather + reduce max
  p_sem = W_sem @ f_isem + b_sem
Returns (p_sem [8, 13, 4096], e_ins [8, 5, 4096]) matching the reference.
"""

from contextlib import ExitStack

import numpy as np

import concourse.bacc as bacc
import concourse.bass as bass
import concourse.mybir as mybir
import concourse.tile as tile
from concourse import bass_utils

B, N = 8, 4096
SEM_IN, SEM_OUT = 128, 13
INS_IN, INS_OUT = 128, 5
K = 30
KS = 32                  # gather slots per query: 30 neighbors + 2 self pads
P = 128
NTILES = N // P          # 32 row tiles
NEG_INF = -3.0e38
AF = mybir.ActivationFunctionType

_CACHE = {}
PROFILE_MODE = False   # skip manual dg_sem (TimelineSim cannot model it)


def build_kernel():
    nc = bacc.Bacc("TRN2", target_bir_lowering=False, debug=False, num_devices=8)

    dt = mybir.dt.float32
    t = {}
    for name, shape in [
        ("f_sem", [P, N]), ("f_ins", [P, N]), ("f_semT", [N, P]),
        ("W_effT", [P, P]), ("b_eff", [P, 1]),
        ("W_insT", [P, INS_OUT]), ("b_ins", [INS_OUT, 1]),
        ("W_semT", [P, SEM_OUT]), ("b_sem", [SEM_OUT, 1]),
        ("ident", [P, P]), ("p_col", [P, 1]),
    ]:
        t[name] = nc.dram_tensor(name, shape, dt, kind="ExternalInput").ap()
    t["p_sem"] = nc.dram_tensor("p_sem", [SEM_OUT, N], dt, kind="ExternalOutput").ap()
    t["e_out"] = nc.dram_tensor("e_out", [INS_OUT, N], dt, kind="ExternalOutput").ap()

    with tile.TileContext(nc) as tc:
        _body(tc, t)
    nc.compile()
    return nc


def _topk_brute(tc, pool, dist_sb, idx):
    """Top-32 values' indices per row via 4 rounds of max/max_index/match_replace.
    First 30 slots of idx are the top-30 neighbor indices."""
    nc = tc.nc
    vals8 = pool.tile([P, 8], mybir.dt.float32, tag="vals8")
    for r in range(4):
        nc.vector.max(out=vals8[:], in_=dist_sb[:])
        nc.vector.max_index(out=idx[:, r * 8:(r + 1) * 8],
                            in_max=vals8[:], in_values=dist_sb[:])
        if r < 3:
            nc.vector.match_replace(out=dist_sb[:], in_to_replace=vals8[:],
                                    in_values=dist_sb[:], imm_value=NEG_INF)


def _body(tc, t):
    nc = tc.nc
    dt = mybir.dt.float32
    CH = 512                      # matmul free-dim chunk
    NCH = N // CH                 # 8

    with ExitStack() as ctx:
        const = ctx.enter_context(tc.tile_pool(name="const", bufs=1))
        big = ctx.enter_context(tc.tile_pool(name="big", bufs=1))

        # ---- persistent tiles ----
        e_nb = big.tile([INS_OUT, N], dt)        # e without bias (for distances)
        aug_l = big.tile([7, N], dt)
        aug_r = big.tile([7, N], dt)
        f_isem = big.tile([P, N], dt)

        W_effT = const.tile([P, P], dt)
        b_eff = const.tile([P, 1], dt)
        W_insT = const.tile([P, INS_OUT], dt)
        b_ins = const.tile([INS_OUT, 1], dt)
        W_semT = const.tile([P, SEM_OUT], dt)
        b_sem = const.tile([SEM_OUT, 1], dt)
        identity = const.tile([P, P], dt)
        p_col = const.tile([P, 1], dt)
        ones5 = const.tile([INS_OUT, 1], dt)
        for sb, nm in ((W_effT, "W_effT"), (b_eff, "b_eff"), (W_insT, "W_insT"),
                       (b_ins, "b_ins"), (W_semT, "W_semT"), (b_sem, "b_sem"),
                       (identity, "ident"), (p_col, "p_col")):
            nc.sync.dma_start(out=sb[:], in_=t[nm][:])
        nc.vector.memset(ones5[:], 1.0)

        # ---- prep phase: f_sins = f_ins + relu(W_eff @ f_sem + b_eff); e = W_ins @ f_sins
        with tc.tile_pool(name="prep_big", bufs=1) as prep_big, \
             tc.tile_pool(name="prep_ps", bufs=2, space="PSUM") as prep_ps, \
             tc.tile_pool(name="prep_sb", bufs=2) as prep_sb:
            f_sem = prep_big.tile([P, N], dt)
            f_ins = prep_big.tile([P, N], dt)
            f_sins = prep_big.tile([P, N], dt)
            nc.sync.dma_start(out=f_sem[:], in_=t["f_sem"][:])
            nc.sync.dma_start(out=f_ins[:], in_=t["f_ins"][:])
            for j in range(NCH):
                sl = slice(j * CH, (j + 1) * CH)
                ps_a = prep_ps.tile([P, CH], dt, tag="ps_a")
                nc.tensor.matmul(out=ps_a[:], lhsT=W_effT[:], rhs=f_sem[:, sl])
                ad = prep_sb.tile([P, CH], dt, tag="ad")
                nc.scalar.activation(out=ad[:], in_=ps_a[:], func=AF.Relu,
                                     bias=b_eff[:], scale=1.0)
                nc.vector.tensor_add(out=f_sins[:, sl], in0=ad[:], in1=f_ins[:, sl])
                ps_e = prep_ps.tile([INS_OUT, CH], dt, tag="ps_e")
                nc.tensor.matmul(out=ps_e[:], lhsT=W_insT[:], rhs=f_sins[:, sl])
                nc.vector.tensor_copy(out=e_nb[:, sl], in_=ps_e[:])
                eo = prep_sb.tile([INS_OUT, CH], dt, tag="eo")
                nc.scalar.activation(out=eo[:], in_=ps_e[:], func=AF.Identity,
                                     bias=b_ins[:], scale=1.0)
                nc.sync.dma_start(out=t["e_out"][:, sl], in_=eo[:])

            # ---- sq = sum_c e^2; build aug_l [7, N], aug_r [7, N] ----
            # engine ops can only write partition-0-aligned APs, so stage the
            # scalar rows in [1, N] tiles and DMA them into rows 5/6.
            sq_sb = prep_big.tile([1, N], dt, tag="sq_sb")
            neg_sq = prep_big.tile([1, N], dt, tag="neg_sq")
            const_row = prep_big.tile([1, N], dt, tag="const_row")
            for j in range(NCH):
                sl = slice(j * CH, (j + 1) * CH)
                e2 = prep_sb.tile([INS_OUT, CH], dt, tag="e2")
                nc.vector.tensor_mul(out=e2[:], in0=e_nb[:, sl], in1=e_nb[:, sl])
                ps_sq = prep_ps.tile([1, CH], dt, tag="ps_sq")
                nc.tensor.matmul(out=ps_sq[:], lhsT=ones5[:], rhs=e2[:])
                nc.vector.tensor_copy(out=sq_sb[:, sl], in_=ps_sq[:])
                nc.scalar.activation(out=neg_sq[:, sl], in_=ps_sq[:],
                                     func=AF.Copy, bias=0.0, scale=-1.0)
            nc.vector.tensor_scalar_mul(aug_l[0:5, :], e_nb[:], 2.0)
            nc.vector.tensor_copy(out=aug_r[0:5, :], in_=e_nb[:])
            nc.sync.dma_start(out=aug_l[5:6, :], in_=neg_sq[:])
            nc.sync.dma_start(out=aug_r[6:7, :], in_=sq_sb[:])
            nc.vector.memset(const_row[:], -1.0)
            nc.sync.dma_start(out=aug_l[6:7, :], in_=const_row[:])
            nc.vector.memset(const_row[:], 1.0)
            nc.sync.dma_start(out=aug_r[5:6, :], in_=const_row[:])

        # ---- per-row-tile: dist, top-k, gather, reduce ----
        QCH = 1024                # psum chunk: 2 banks
        NQ = N // QCH             # 4
        dg_sem = tc.ctx.enter_context(nc.semaphore("dg_sem"))
        with tc.tile_pool(name="dist_ps", bufs=2, space="PSUM") as dist_ps, \
             tc.tile_pool(name="tp_ps", bufs=2, space="PSUM") as tp_ps, \
             tc.tile_pool(name="t2_ps", bufs=2, space="PSUM") as t2_ps, \
             tc.tile_pool(name="dist_sb", bufs=2) as dist_sb_pool, \
             tc.tile_pool(name="work", bufs=2) as work_pool:
            for i in range(NTILES):
                rs = slice(i * P, (i + 1) * P)
                dist_sb = dist_sb_pool.tile([P, N], dt, tag="dist")
                for q in range(NQ):
                    ps_d = dist_ps.tile([P, QCH], dt, tag="ps_d")
                    for h in range(QCH // CH):
                        sl_out = slice(h * CH, (h + 1) * CH)
                        sl_in = slice(q * QCH + h * CH, q * QCH + (h + 1) * CH)
                        nc.tensor.matmul(out=ps_d[:, sl_out], lhsT=aug_l[:, rs],
                                         rhs=aug_r[:, sl_in])
                    nc.scalar.copy(out=dist_sb[:, q * QCH:(q + 1) * QCH], in_=ps_d[:])

                # top-30 indices per query -> idxf (f32), slots 30/31 = self
                idx = work_pool.tile([P, KS], mybir.dt.uint32, tag="idx")
                _topk_brute(tc, work_pool, dist_sb, idx)
                idxf = work_pool.tile([P, KS], dt, tag="idxf")
                nc.vector.tensor_copy(out=idxf[:, :K], in_=idx[:, :K])
                nc.vector.tensor_scalar_add(idxf[:, K:KS],
                                            p_col[:].to_broadcast([P, KS - K]),
                                            float(i * P))

                # build the 16-wrapped flat index layout for dma_gather:
                # idxw[p, 8*ii + g] = idx[16*g + p%16, ii]
                ps_t1 = tp_ps.tile([KS, P], dt, tag="tp")
                nc.tensor.transpose(out=ps_t1[:], in_=idxf[:], identity=identity[:])
                idxT = work_pool.tile([KS, P], dt, tag="idxT")
                nc.scalar.copy(out=idxT[:], in_=ps_t1[:])
                idxw16 = work_pool.tile([16, KS * 8], mybir.dt.int16, tag="idxw16")
                idxw16_v = idxw16[:].rearrange("p (a g) -> p a g", g=8)
                for g in range(8):
                    ps_t2 = t2_ps.tile([16, KS], dt, tag="t2")
                    nc.tensor.transpose(out=ps_t2[:], in_=idxT[:, 16 * g:16 * (g + 1)],
                                        identity=identity[:KS, :KS])
                    nc.scalar.copy(out=idxw16_v[:, :, g], in_=ps_t2[:])
                idxw = work_pool.tile([P, KS * 8], mybir.dt.int16, tag="idxw")
                for h in range(8):
                    nc.sync.dma_start(out=idxw[16 * h:16 * (h + 1), :], in_=idxw16[:])

                # bulk-gather f_semT rows: 4 x 1024-index dma_gather
                gath = work_pool.tile([P, KS, P], dt, tag="gath")
                for si in range(4):
                    dg = nc.gpsimd.dma_gather(
                        out_ap=gath[:, si * (KS // 4):(si + 1) * (KS // 4), :],
                        in_ap=t["f_semT"][:],
                        idxs_ap=idxw[:, si * (KS * 2):(si + 1) * (KS * 2)],
                        num_idxs=1024, num_idxs_reg=1024, elem_size=P,
                    )
                    if not PROFILE_MODE:
                        dg.then_inc(dg_sem, 16)
                fit = work_pool.tile([P, P], dt, tag="fit")
                if PROFILE_MODE:
                    nc.vector.tensor_reduce(out=fit[:],
                                            in_=gath[:].rearrange("p k c -> p c k"),
                                            axis=mybir.AxisListType.X,
                                            op=mybir.AluOpType.max)
                else:
                    with tc.tile_critical():
                        nc.vector.wait_ge(dg_sem, 64 * (i + 1))
                        nc.vector.tensor_reduce(out=fit[:],
                                                in_=gath[:].rearrange("p k c -> p c k"),
                                                axis=mybir.AxisListType.X,
                                                op=mybir.AluOpType.max)
                # transpose back to [channel, point]
                ps_t = tp_ps.tile([P, P], dt, tag="tp")
                nc.tensor.transpose(out=ps_t[:], in_=fit[:], identity=identity[:])
                nc.scalar.copy(out=f_isem[:, rs], in_=ps_t[:])

        # ---- p_sem = W_sem @ f_isem + b_sem ----
        with tc.tile_pool(name="out_ps", bufs=2, space="PSUM") as out_ps, \
             tc.tile_pool(name="out_sb", bufs=2) as out_sb:
            for j in range(NCH):
                sl = slice(j * CH, (j + 1) * CH)
                ps_p = out_ps.tile([SEM_OUT, CH], dt, tag="ps_p")
                nc.tensor.matmul(out=ps_p[:], lhsT=W_semT[:], rhs=f_isem[:, sl])
                po = out_sb.tile([SEM_OUT, CH], dt, tag="po")
                nc.scalar.activation(out=po[:], in_=ps_p[:], func=AF.Identity,
                                     bias=b_sem[:], scale=1.0)
                nc.sync.dma_start(out=t["p_sem"][:, sl], in_=po[:])


def host_prep(f_sem, f_ins, W_adapt, b_adapt, gamma_adapt, beta_adapt,
              W_ins, b_ins, W_sem, b_sem):
    """Fold BN into conv weights and build per-core input maps."""
    f_sem = np.asarray(f_sem, dtype=np.float32)
    f_ins = np.asarray(f_ins, dtype=np.float32)
    W_eff = (np.asarray(gamma_adapt)[:, None] * np.asarray(W_adapt)).astype(np.float32)
    b_eff = (np.asarray(gamma_adapt) * np.asarray(b_adapt)
             + np.asarray(beta_adapt)).astype(np.float32)
    shared = {
        "W_effT": np.ascontiguousarray(W_eff.T),
        "b_eff": b_eff.reshape(P, 1),
        "W_insT": np.ascontiguousarray(np.asarray(W_ins, np.float32).T),
        "b_ins": np.asarray(b_ins, np.float32).reshape(INS_OUT, 1),
        "W_semT": np.ascontiguousarray(np.asarray(W_sem, np.float32).T),
        "b_sem": np.asarray(b_sem, np.float32).reshape(SEM_OUT, 1),
        "ident": np.eye(P, dtype=np.float32),
        "p_col": np.arange(P, dtype=np.float32).reshape(P, 1),
    }
    in_maps = []
    for b in range(B):
        m = dict(shared)
        m["f_sem"] = np.ascontiguousarray(f_sem[b])
        m["f_ins"] = np.ascontiguousarray(f_ins[b])
        m["f_semT"] = np.ascontiguousarray(f_sem[b].T)
        in_maps.append(m)
    return in_maps


def kernel(f_sem, f_ins, W_adapt, b_adapt, gamma_adapt, beta_adapt,
           W_ins, b_ins, W_sem, b_sem, _trace=False):
    in_maps = host_prep(f_sem, f_ins, W_adapt, b_adapt, gamma_adapt, beta_adapt,
                        W_ins, b_ins, W_sem, b_sem)
    if "nc" not in _CACHE:
        _CACHE["nc"] = build_kernel()
    nc = _CACHE["nc"]
    res = bass_utils.run_bass_kernel_spmd(
        nc, in_maps, core_ids=list(range(B)), trace=_trace,
    )
    p_sem = np.stack([r["p_sem"] for r in res.results])
    e_ins = np.stack([r["e_out"] for r in res.results])
    _CACHE["last_result"] = res
    return p_sem, e_ins
